# revision 1
# baseline (speedup 1.0000x reference)
"""Trainium2 Bass kernel for nn_DGBasedVonMisesFisherKLD.

Computes okl = mean_j [ logsumexp_i (log_C_kappa + kappa * mu_n[i]@z2[j]) - log A ] - log_C_zero
where mu_n is row-normalized mu [2048, 32], z2 is z reshaped to [65536, 32].

Strategy (per spec sharding hint): shard the j axis (65536) across 8 cores.
mu is replicated. Each core computes, for its 8192 j's:
    S_j = sum_i exp(kappa*m_ij - kappa)   (constant shift is safe: m <= 1)
    partial = sum_j ln(S_j)
Pipeline per 128-j tile:
  TensorE: 2x row-group-packed fp32r matmuls (K padded to 64; the -kappa
           shift rides as an extra contraction row) -> PSUM [128, 2048]
  exp+sum over the 2048 i's: split between ScalarE (native Exp with fused
           accumulate) and VectorE (custom DVE op: exp(y) ~ (1+t+t^2/2)^1024,
           t=y/1024, via 10 chained squarings, fused ADD accumulate)
  final ln+sum on ScalarE; host combines 8 tiny partials.
"""

import math
import os
import sys

import numpy as np

if "/opt/trn_rl_repo" not in sys.path:
    sys.path.insert(0, "/opt/trn_rl_repo")

BATCH = 2048
DIM = 32
N_SAMPLES = 32
N_CORES = 8
J_PER_CORE = BATCH * N_SAMPLES // N_CORES  # 8192
N_JT = J_PER_CORE // 128  # 64 j-tiles of 128
I_CHUNK = 512
N_IC = BATCH // I_CHUNK  # 4 i-chunks of 512

# 3 of every 7 j-tiles are reduced on VectorE (custom exp) instead of ScalarE
DVE_MODE = int(os.environ.get("BASS_DVE_MODE", "1"))  # 0 = all-ScalarE

_CACHE = {}
_DVE_OPS = {}


# ---- fallback constants (normally passed in as inputs) ----
def _log_iv(v, x, n_terms=300):
    ks = np.arange(n_terms)
    lg = np.array([math.lgamma(k + 1.0) + math.lgamma(v + k + 1.0) for k in ks])
    logt = (v + 2 * ks) * np.log(x / 2.0) - lg
    m = logt.max()
    return float(m + np.log(np.exp(logt - m).sum()))


def _log_C_d(kappa, d):
    v = d / 2.0 - 1.0
    if kappa == 0.0:
        return float(math.lgamma(d / 2.0) - math.log(2.0) - (d / 2.0) * math.log(math.pi))
    return float(
        v * math.log(kappa) - (d / 2.0) * math.log(2.0 * math.pi) - _log_iv(v, kappa)
    )


def _register_dve_exp_ops():
    """Register two chained custom DVE ops computing exp(y + shift) for
    raw logits y = kappa*m in [-100, 100], shift = -kappa:
    op1: t = y*C0 + C2 (C0=1/512, C2=-kappa/512); u = 1 + t + t^2/2; u^4
    op2: (.)^128 (7 squarings) with fused ADD-reduction to accum_out.
    Result = (1 + t + t^2/2)^512 ~ exp(y-kappa), rel err ~ |y-k|^3/(6*512^2):
    ~1.4e-3 at the dominant logsumexp terms -> ~3e-5 relative on the final
    mean, fine for this loss."""
    if _DVE_OPS:
        return _DVE_OPS
    from concourse import dve_ops as DO
    from concourse.dve_spec import AluOp, C0, C1, C2, One, Spec, Src0, lower, sq
    from concourse.dve_uop import DveOpSpec

    t = Src0 * C0 + C2
    u = (One + t) + sq(t) * C1
    v = sq(sq(u))
    spec1 = Spec(
        body=v,
        reference=lambda in0, in1, c0, c1, c2: (
            1.0
            + (in0 * c0 + c2)
            + np.square(in0 * c0 + c2) * c1
        )
        ** 4,
    )

    w = Src0
    for _ in range(7):
        w = sq(w)
    spec2 = Spec(
        body=w,
        accum=AluOp.ADD,
        reference=lambda in0, in1, c0, c1, c2: (
            in0 ** 128,
            (in0 ** 128).sum(axis=-1, keepdims=True),
        ),
    )

    from concourse.dve_ops import has_src1

    ops = {}
    for name, spec in (("EXP_PT1_ANT", spec1), ("EXP_PT2_ANT", spec2)):
        if name in DO._SUB_OPCODE_FOR_NAME:
            ops[name] = next(o for o in DO.OPS if o.name == name)
            continue
        shas = {}
        for ver in ("v3", "v4"):
            try:
                s = DveOpSpec(
                    name=name,
                    opcode=DO._CUSTOM_DVE_ROW_BASE + len(DO.OPS),
                    uops=lower(spec, ver=ver),
                    rd1_en=has_src1(spec),
                )
                shas[ver] = s.sha(ver)
            except Exception:
                pass
        op = DO.DveOp(name, spec, subdim=False, uops_sha=shas)
        DO.OPS.append(op)
        DO._SUB_OPCODE_FOR_NAME[name] = (
            DO._CUSTOM_DVE_ROW_BASE + len(DO.OPS) - 1
        )
        DO.CUSTOM_DVE_SPECS[name] = spec
        ops[name] = op
    _DVE_OPS.update(ops)
    return _DVE_OPS


def _build_nc(kappa: float, mm_dtype: str, dve_mode: int):
    """Build the single-core SPMD Bass program (same NEFF on all 8 cores)."""
    import concourse.tile as tile
    from concourse import bacc, mybir

    f32 = mybir.dt.float32
    f32r = mybir.dt.float32r
    mm_dt = f32r if mm_dtype == "f32r" else f32
    AF = mybir.ActivationFunctionType

    if dve_mode:
        dve_ops = _register_dve_exp_ops()
        op1 = dve_ops["EXP_PT1_ANT"]
        op2 = dve_ops["EXP_PT2_ANT"]
    # t%3==1 (not ==2) so the last DVE tile lands at t=61: the slower DVE
    # path drains two tiles before loop end and the final ln overlaps it
    dve_tiles = [t for t in range(N_JT) if dve_mode and t % 3 == 1]
    act_tiles = [t for t in range(N_JT) if t not in dve_tiles]

    nc = bacc.Bacc("TRN2", target_bir_lowering=False, debug=False, num_devices=N_CORES)

    # zT = z2^T [32, J]; replicated on-device into the 4 PE row-group strips
    # for 4x-packed K=32 matmuls (tile_position row tiling).
    w_dt = mm_dt
    zT_d = nc.dram_tensor("zT", [DIM, J_PER_CORE], w_dt, kind="ExternalInput").ap()
    muT_d = nc.dram_tensor("muT", [DIM, BATCH], f32, kind="ExternalInput").ap()
    out_d = nc.dram_tensor("out", [128, 2], f32, kind="ExternalOutput").ap()

    with tile.TileContext(nc) as tc:
        with (
            tc.tile_pool(name="big", bufs=1) as big,
            tc.tile_pool(name="small", bufs=1) as small,
            tc.tile_pool(name="scr", bufs=2) as scr,
        ):
            # ---- loads: muT first (it heads the prologue critical path),
            # then the 4 zT strip replicas ----
            # split strip loads across both HWDGE issue queues (sync+scalar)
            muT = big.tile([128, BATCH], f32)
            for g in range(4):
                eng = nc.sync if g % 2 == 0 else nc.scalar
                eng.dma_start(muT[32 * g : 32 * (g + 1), :], muT_d[:])
            zT = big.tile([128, J_PER_CORE], w_dt)
            for g in range(4):
                eng = nc.sync if g % 2 == 0 else nc.scalar
                eng.dma_start(zT[32 * g : 32 * (g + 1), :], zT_d[:])

            # ones in f32r so the prologue matmuls run at f32r rate instead
            # of fp32's two-instruction half-speed emulation; memset can't
            # write f32r, so memset f32 then retag via a tiny DVE copy
            ones_f32 = small.tile([DIM, 1], f32)
            nc.vector.memset(ones_f32[:], 1.0)
            ones_k32 = small.tile([DIM, 1], mm_dt)
            nc.vector.tensor_copy(ones_k32[:], ones_f32[:])
            ones1_f32 = small.tile([1, 128], f32)
            nc.vector.memset(ones1_f32[:], 1.0)
            ones_k1 = small.tile([1, 128], mm_dt)
            nc.vector.tensor_copy(ones_k1[:], ones1_f32[:])
            bias_negk = small.tile([128, 1], f32)
            nc.vector.memset(bias_negk[:], -kappa)

            # prefetch the exp/ln ACT table set at t~0 (concurrent with the
            # input DMAs) so the prologue Ln doesn't stall ~2.7us on the
            # PSEUDO_LOAD_ACT_FUNC_SET, and both funcs land in one set
            warm_act = small.tile([DIM, 1], f32)
            nc.scalar.activation(warm_act[:], ones_k32[:], AF.Exp)
            nc.scalar.activation(warm_act[:], warm_act[:], AF.Ln)

            # ---- mu normalization (in transposed layout), scaled by kappa ----
            musq = big.tile([DIM, BATCH], mm_dt)
            nc.vector.tensor_tensor(
                out=musq[:],
                in0=muT[0:DIM, :],
                in1=muT[0:DIM, :],
                op=mybir.AluOpType.mult,
            )
            muS = big.tile([128, BATCH], mm_dt)  # kappa*mu_n^T in 4 strips
            acc_a = small.tile([128, max(len(act_tiles), 1)], f32)
            acc_d = small.tile([128, max(len(dve_tiles), 1)], f32)

            with tc.tile_pool(name="pp", bufs=1, space="PSUM") as pp:
                # sum of squares per i: ones^T @ musq -> [1, 2048]
                ss = pp.tile([1, BATCH], f32, tag="pre")
                for k in range(N_IC):
                    nc.tensor.matmul(
                        ss[:, k * I_CHUNK : (k + 1) * I_CHUNK],
                        ones_k32[:],
                        musq[:, k * I_CHUNK : (k + 1) * I_CHUNK],
                        start=True,
                        stop=True,
                    )
                # 1 / ||mu_i|| = exp(-0.5*ln(ss)); kappa folded in below
                lnss = small.tile([1, BATCH], f32)
                nc.scalar.activation(lnss[:], ss[:], AF.Ln)
                invk = small.tile([1, BATCH], mm_dt)
                nc.scalar.activation(invk[:], lnss[:], AF.Exp, scale=-0.5)
                # broadcast invk across all 128 partitions via K=1 matmul
                bc = pp.tile([128, BATCH], f32, tag="pre")
                for k in range(N_IC):
                    nc.tensor.matmul(
                        bc[:, k * I_CHUNK : (k + 1) * I_CHUNK],
                        ones_k1[:],
                        invk[:, k * I_CHUNK : (k + 1) * I_CHUNK],
                        start=True,
                        stop=True,
                    )
                # muS = (muT * kappa) * (1/||mu_i||) on all 128 partitions
                nc.vector.scalar_tensor_tensor(
                    out=muS[:],
                    in0=muT[:],
                    scalar=float(kappa),
                    in1=bc[:],
                    op0=mybir.AluOpType.mult,
                    op1=mybir.AluOpType.mult,
                )
                # absorber: fold the zT-DMA completion into the PE vector
                # clock early (wait-count hygiene for the main loop)
                warm = pp.tile([1, 16], f32)
                nc.tensor.matmul(
                    warm[:], zT[0:DIM, 0:1], zT[0:DIM, 0:16], start=True, stop=True
                )

            # ---- main loop ----
            ia = 0
            idv = 0
            with tc.tile_pool(name="ps", bufs=2, space="PSUM") as ps:
                for t in range(N_JT):
                    P = ps.tile([128, BATCH], f32)
                    for g in range(4):
                        nc.tensor.matmul(
                            P[:, g * I_CHUNK : (g + 1) * I_CHUNK],
                            zT[32 * g : 32 * (g + 1), t * 128 : (t + 1) * 128],
                            muS[32 * g : 32 * (g + 1), g * I_CHUNK : (g + 1) * I_CHUNK],
                            start=True,
                            stop=True,
                            tile_position=(32 * g, 0),
                        )
                    if t in dve_tiles:
                        s1 = scr.tile([128, BATCH], f32, tag="s1")
                        s2 = scr.tile([128, BATCH], f32, tag="s2")
                        nc.vector._custom_dve(
                            op1,
                            out=s1[:],
                            in0=P[:],
                            s0=1.0 / 512.0,
                            s1=0.5,
                            imm2=-float(kappa) / 512.0,
                        )
                        nc.vector._custom_dve(
                            op2,
                            out=s2[:],
                            in0=s1[:],
                            accum_out=acc_d[:, idv : idv + 1],
                        )
                        idv += 1
                    else:
                        nc.scalar.activation(
                            P[:],
                            P[:],
                            AF.Exp,
                            bias=bias_negk[:],
                            accum_out=acc_a[:, ia : ia + 1],
                        )
                        ia += 1

            # ---- ln(S_j), summed over j-tiles ----
            lnacc_a = small.tile([128, max(len(act_tiles), 1)], f32)
            lnsum = small.tile([128, 2], f32)
            nc.vector.memset(lnsum[:], 0.0)
            nc.scalar.activation(
                lnacc_a[:], acc_a[:], AF.Ln, accum_out=lnsum[:, 0:1]
            )
            if dve_tiles:
                lnacc_d = small.tile([128, len(dve_tiles)], f32)
                nc.scalar.activation(
                    lnacc_d[:], acc_d[:], AF.Ln, accum_out=lnsum[:, 1:2]
                )
            nc.sync.dma_start(out_d[:], lnsum[:])

    nc.finalize()  # Bacc passes: wait-splitting, nop-fusion, act table loads
    return nc


def _get_nc(kappa: float, mm_dtype: str, dve_mode: int = DVE_MODE):
    key = (kappa, mm_dtype, dve_mode)
    if key not in _CACHE:
        _CACHE[key] = _build_nc(kappa, mm_dtype, dve_mode)
    return _CACHE[key]


def _install_trace_hook():
    """The image's antenv lacks axon_hooks; shim it so trace=True can ship
    NTFFs back through libaxon_pjrt.so. Safe no-op on failure."""
    try:
        import types

        import antenv

        if "antenv.axon_hooks" not in sys.modules:
            mod = types.ModuleType("antenv.axon_hooks")
            mod._hook = None
            mod.set_axon_ntff_profile_hook = lambda h: setattr(mod, "_hook", h)
            mod.get_axon_ntff_profile_hook = lambda: mod._hook
            sys.modules["antenv.axon_hooks"] = mod
            antenv.axon_hooks = mod
        hooks = sys.modules["antenv.axon_hooks"]
        if hooks.get_axon_ntff_profile_hook() is None:
            from trn_agent_boot.trn_boot import _ntff_profile_via_ctypes

            hooks.set_axon_ntff_profile_hook(
                _ntff_profile_via_ctypes("/opt/axon/libaxon_pjrt.so")
            )
        return True
    except Exception as e:  # pragma: no cover
        print(f"trace hook install failed: {e}")
        return False


def _run(mu, z, kappa, log_C_kappa, log_C_zero, n_samples, trace=False):
    from concourse.bass_utils import run_bass_kernel_spmd

    if trace:
        trace = _install_trace_hook()

    mu = np.ascontiguousarray(np.asarray(mu, dtype=np.float32))
    z = np.ascontiguousarray(np.asarray(z, dtype=np.float32))
    B, d = mu.shape
    n = int(n_samples)
    assert (B, d, n) == (BATCH, DIM, N_SAMPLES), (B, d, n)

    mm_dtype = os.environ.get("BASS_MM_DTYPE", "f32r")
    nc = _get_nc(float(kappa), mm_dtype)

    muT = np.ascontiguousarray(mu.T)
    rows = B // N_CORES
    in_maps = []
    for c in range(N_CORES):
        zc = z[c * rows : (c + 1) * rows].reshape(-1, d)
        in_maps.append({"zT": np.ascontiguousarray(zc.T), "muT": muT})

    res = run_bass_kernel_spmd(
        nc, in_maps, core_ids=list(range(N_CORES)), trace=trace
    )
    total = sum(float(r["out"].astype(np.float64).sum()) for r in res.results)
    okl = (
        float(log_C_kappa)
        + float(kappa)
        - math.log(B)
        - float(log_C_zero)
        + total / (B * n)
    )
    return np.float32(okl), res


def kernel(
    mu,
    z,
    kappa=100.0,
    log_C_kappa=None,
    log_C_zero=None,
    n_samples=N_SAMPLES,
    **_ignored,
):
    mu = np.asarray(mu)
    if log_C_kappa is None:
        log_C_kappa = _log_C_d(float(kappa), mu.shape[1])
    if log_C_zero is None:
        log_C_zero = _log_C_d(0.0, mu.shape[1])
    okl, _ = _run(mu, z, kappa, log_C_kappa, log_C_zero, n_samples, trace=False)
    return okl

